# revision 28
# baseline (speedup 1.0000x reference)
"""Cross-attention Trainium2 Bass kernel.

Problem (per full input):
    q_in [8, 2048, 1024] f32, k_v [8, 2048, 1024] f32,
    Wq/Wk/Wv [1024, 1024] f32, bq/bk/bv [1024] f32
    q = q_in @ Wq + bq; k = k_v @ Wk + bk; v = k_v @ Wv + bv
    out = softmax(q k^T / sqrt(1024)) v        -> [8, 2048, 1024] f32

Sharding: data-parallel over batch, one batch per NeuronCore (8 cores).

Weight folding (host, input-independent):
    q k^T = (q_in Wq + bq)(k_v Wk + bk)^T
          = q_in (Wq Wk^T) k_v^T  +  [per-i const: softmax-invariant, dropped]
          +  k_v (Wk bq)  +  [const: dropped]
  so with M = Wq Wk^T and g_j = SCALE * k_v[j] . (Wk bq):
    attn = softmax(SCALE * q' k_v^T + g),  q' = q_in M
  The k-projection disappears from the device entirely (2.1 GMAC/core saved)
  and the attention keys are the *raw* fp16 k_v (less accumulated rounding
  than a computed k).  M is rounded to fp16 once, same 3e-4 as any weight.

Per-core algorithm (I = J = 2048, E = D = 1024, P = 128):
  - Host pre-transposes the activations to [E, I]/[E, J] and casts to fp16.
  - q'-projection: q'T[d,i] computed with the M chunk stationary (output
    comes out transposed, exactly the layout the sim matmul needs);
    v[j,e] computed with the k_vT chunk stationary.
  - Attention: simT[j,i] = k_vT^T q'T accumulated over d in PSUM; exp on the
    ACT engine with the 1/sqrt(E) scale and per-j bias g fused; PV
    accumulates sum_j expT[j,i] v[j,e] over all j in PSUM (unnormalized);
    the softmax denominator is computed entirely off the PE (DVE partial
    sums of the exp tiles, a DMA transpose to [i,j] layout, and a DVE
    free-axis reduce); a per-partition reciprocal multiply normalizes at
    eviction.
  - exp is computed without max subtraction: sim ~ N(0,1) for this
    problem's distribution, so exp() stays comfortably inside fp16/fp32
    range and softmax is shift-invariant anyway.
"""

import numpy as np
from contextlib import ExitStack

import concourse.bass as bass
import concourse.mybir as mybir
import concourse.tile as tile
from concourse import bacc
from concourse.bass_utils import run_bass_kernel_spmd

B = 8
I = 2048  # query positions per batch
J = 2048  # key positions per batch
E = 1024  # embed dim
P = 128
EC = E // P  # 8 contraction chunks
NJC = J // P  # 16 key chunks
SCALE = float(E) ** -0.5

F16 = mybir.dt.float16
F32 = mybir.dt.float32

# i-block size for the attention phase (sim moving free dim).  512 (a full
# PSUM bank per sim tile) with the two-pass block structure: all 16 sim/exp
# tiles of a block first, then PV swept per i-subtile.  PSUM budget:
# 2 sim (double-buffer) + 4 PV (2 e-halves x 2 subtiles in flight) + 2 den.
IB = 512

# Module-level knobs test.py may override before the first kernel() call.
_RUN_KWARGS: dict = {}
LAST_RESULTS = None

_NC_CACHE: dict = {}


def _build():
    nc = bacc.Bacc("TRN2", target_bir_lowering=False, debug=False)

    q_inT = nc.dram_tensor("q_inT", [E, I], F16, kind="ExternalInput")
    k_vT = nc.dram_tensor("k_vT", [E, J], F16, kind="ExternalInput")
    M_d = nc.dram_tensor("M", [E, E], F16, kind="ExternalInput")
    Wv_d = nc.dram_tensor("Wv", [E, E], F16, kind="ExternalInput")
    bv_bc = nc.dram_tensor("bv_bc", [P, E], F32, kind="ExternalInput")
    g_d = nc.dram_tensor("g_t", [P, NJC], F32, kind="ExternalInput")
    out_d = nc.dram_tensor("out", [I, E], F32, kind="ExternalOutput")

    with tile.TileContext(nc) as tc, ExitStack() as ctx:
        const = ctx.enter_context(tc.tile_pool(name="const", bufs=1))
        bv_sb = const.tile([P, E], F32, tag="bv")
        g_sb = const.tile([P, NJC], F32, tag="g")

        # PE warmup: the HAM clock gate keeps the PE at 1.2 GHz until it has
        # seen ~3.4us of sustained matmul activity.  The first ~10us of the
        # kernel are DMA-bound anyway, so burn them on dummy matmuls over a
        # memset tile — by the time real operands land, the PE runs at 2.4.
        with ExitStack() as wu:
            wpool_ = wu.enter_context(tc.tile_pool(name="warm", bufs=1))
            wps_ = wu.enter_context(
                tc.tile_pool(name="warm_ps", bufs=1, space="PSUM")
            )
            wsrc = wpool_.tile([P, 256], F16)
            nc.vector.memset(wsrc[:], 0.0)
            wps = wps_.tile([P, 256], F32)
            for _ in range(20):
                nc.tensor.matmul(
                    wps[:], wsrc[:, :P], wsrc[:], start=True, stop=True
                )

        # Persistent fp16 operands for the attention phase.
        # q'T/k_vT: chunk d lives at [:, d*I + i]  (layout [d, i] / [d, j])
        # v:        chunk jc lives at [:, jc*E + e] (layout [j, e])
        persist = ctx.enter_context(tc.tile_pool(name="persist", bufs=1))
        qT_sb = persist.tile([P, EC * I], F16, tag="qT")
        kv_sb = persist.tile([P, EC * J], F16, tag="kv")
        v_sb = persist.tile([P, NJC * E], F16, tag="v")

        # Attention-phase SBUF pools are allocated up front, NOT scoped after
        # the projection pools: if they reused the weight/x-stream SBUF bytes
        # the allocator would add a WAR edge making the first exp wait on the
        # last v-projection matmul (~0.8us PE stall at the phase boundary).
        exp_pool = ctx.enter_context(tc.tile_pool(name="exp", bufs=18))
        out_pool = ctx.enter_context(tc.tile_pool(name="outsb", bufs=6))
        small = ctx.enter_context(tc.tile_pool(name="small", bufs=4))
        # Softmax-denominator staging: exp tiles are summed over key chunks
        # on the DVE, DMA-transposed to [i-partition, j] layout, and reduced
        # on the DVE — the PE never touches the denominator.
        acc_pool = ctx.enter_context(tc.tile_pool(name="dacc", bufs=2))
        dent_pool = ctx.enter_context(tc.tile_pool(name="dent", bufs=4))
        # sim PSUM pool allocated before the projection pool so it sits on
        # banks the projections never touch — the first sim matmuls at the
        # phase boundary then carry no PSUM WAR dependency on the last
        # v-projection evictions.
        sim_ps_pool = ctx.enter_context(
            tc.tile_pool(name="sim_ps", bufs=2, space="PSUM")
        )

        # ---------------- phase A/B: projections ----------------
        with ExitStack() as ab:
            wpool = ab.enter_context(tc.tile_pool(name="wpool", bufs=1))
            # Both weight matrices in one tile: matrix w chunk e at
            # [:, w*E*EC + e*E + d]   ([128, 16384] f16 = 32KB/partition).
            w_sb = wpool.tile([P, 2 * EC * E], F16, tag="W")
            w_off = {"M": 0, "Wv": EC * E}

            # Each dma_start costs ~600ns of dispatch on its queue engine, so
            # the startup-critical input stream alternates between the two
            # HWDGE dispatchers (Sync and ACT) to halve the descriptor-issue
            # latency.  ACT is otherwise idle during the DMA-bound start.
            _dma_eng = [nc.sync, nc.scalar]
            _dma_ctr = [0]

            def in_dma(dst, src):
                eng = _dma_eng[_dma_ctr[0] & 1]
                _dma_ctr[0] += 1
                eng.dma_start(dst, src)

            def load_w_chunk(w, wd, e, dh_range=(0, 2)):
                # dh splits each weight chunk into d-halves so the DMA stream
                # can prioritize the columns the first PSUM groups need.
                for dh in range(*dh_range):
                    in_dma(
                        w_sb[:, w_off[w] + e * E + dh * 512
                             : w_off[w] + e * E + (dh + 1) * 512],
                        wd.ap()[e * P : (e + 1) * P, dh * 512 : (dh + 1) * 512],
                    )

            xpool = ab.enter_context(tc.tile_pool(name="xpool", bufs=2))
            ppool = ab.enter_context(
                tc.tile_pool(name="proj_ps", bufs=4, space="PSUM")
            )

            H = 1024  # half of the i range handled per streamed xT tile

            def load_q_half(h, with_m):
                # First half: DMA order matches the group order below — M
                # first-halves paired with the first 512-wide i-slice of each
                # q chunk (the exact data the first four PSUM groups need),
                # then the second i-slices, then the M second-halves.
                xh = xpool.tile([P, EC * H], F16, tag="xT")
                for e in range(EC):
                    if with_m:
                        load_w_chunk("M", M_d, e, dh_range=(0, 1))
                    in_dma(
                        xh[:, e * H : e * H + 512],
                        q_inT.ap()[e * P : (e + 1) * P, h * H : h * H + 512],
                    )
                    if h == 0 and with_m and e == 0:
                        in_dma(bv_sb[:], bv_bc.ap())
                        in_dma(g_sb[:], g_d.ap())
                for e in range(EC):
                    in_dma(
                        xh[:, e * H + 512 : (e + 1) * H],
                        q_inT.ap()[e * P : (e + 1) * P,
                                   h * H + 512 : (h + 1) * H],
                    )
                if with_m:
                    # second d-halves after the critical set
                    for e in range(EC):
                        load_w_chunk("M", M_d, e, dh_range=(1, 2))
                return xh

            def proj_q(xh, h, order):
                # q'T[d, n] = sum_e M[e,d] q_in[n,e], n in this half
                for d, ib in order:
                    ps = ppool.tile([P, 512], F32, tag="proj")
                    for e in range(EC):
                        nc.tensor.matmul(
                            ps[:],
                            w_sb[:, w_off["M"] + e * E + d * P
                                 : w_off["M"] + e * E + (d + 1) * P],
                            xh[:, e * H + ib * 512 : e * H + (ib + 1) * 512],
                            start=(e == 0),
                            stop=(e == EC - 1),
                        )
                    nc.scalar.activation(
                        qT_sb[:, d * I + h * H + ib * 512
                              : d * I + h * H + (ib + 1) * 512],
                        ps[:],
                        mybir.ActivationFunctionType.Copy,
                    )

            # first-half group order follows DMA arrival: (ib0, d<4) needs
            # only the M first-halves + first i-slices; (ib1, d<4) adds the
            # second i-slices; d>=4 waits on the M second-halves.
            order0 = ([(d, 0) for d in range(4)] + [(d, 1) for d in range(4)]
                      + [(d, ib) for d in range(4, EC) for ib in range(2)])
            order1 = [(d, ib) for d in range(EC) for ib in range(H // 512)]
            xh = load_q_half(0, True)
            proj_q(xh, 0, order0)
            xh = load_q_half(1, False)
            proj_q(xh, 1, order1)

            # keys (raw) + Wv stream in while the PE chews on q'
            for e in range(EC):
                in_dma(
                    kv_sb[:, e * J : (e + 1) * J],
                    k_vT.ap()[e * P : (e + 1) * P, :],
                )
                load_w_chunk("Wv", Wv_d, e)

            # v[j, e] = sum_e' k_v[j, e'] Wv[e', e] + bv[e]
            for jc in range(NJC):
                for eh in range(E // 512):
                    ps = ppool.tile([P, 512], F32, tag="proj")
                    for e in range(EC):
                        nc.tensor.matmul(
                            ps[:],
                            kv_sb[:, e * J + jc * P : e * J + (jc + 1) * P],
                            w_sb[:, w_off["Wv"] + e * E + eh * 512
                                 : w_off["Wv"] + e * E + (eh + 1) * 512],
                            start=(e == 0),
                            stop=(e == EC - 1),
                        )
                    nc.vector.tensor_add(
                        v_sb[:, jc * E + eh * 512 : jc * E + (eh + 1) * 512],
                        ps[:],
                        bv_sb[:, eh * 512 : (eh + 1) * 512],
                    )

        # ---------------- phase C: attention ----------------
        with ExitStack() as c:
            pv_ps_pool = c.enter_context(
                tc.tile_pool(name="pv_ps", bufs=4, space="PSUM")
            )
            NSUB = IB // P  # i-subtiles per block

            for ib in range(I // IB):
                i0 = ib * IB

                def emit_sim(jc):
                    sim = sim_ps_pool.tile([P, IB], F32, tag="sim",
                                           name=f"sim_{ib}_{jc}")
                    for d in range(EC):
                        nc.tensor.matmul(
                            sim[:],
                            kv_sb[:, d * J + jc * P : d * J + (jc + 1) * P],
                            qT_sb[:, d * I + i0 : d * I + i0 + IB],
                            start=(d == 0),
                            stop=(d == EC - 1),
                        )
                    return sim

                # All 16 sim tiles first (exp on ACT trails one tile behind),
                # then PV grouped by i-subtile so each subtile's
                # reciprocal+eviction overlaps the next subtile's PV matmuls.
                exps = []
                acc = acc_pool.tile([P, IB], F32, tag="acc")
                for jc in range(NJC):
                    sim = emit_sim(jc)
                    expT = exp_pool.tile([P, IB], F16, tag="expT")
                    nc.scalar.activation(
                        expT[:], sim[:], mybir.ActivationFunctionType.Exp,
                        scale=SCALE, bias=g_sb[:, jc : jc + 1],
                    )
                    exps.append(expT)
                    # denominator partial sum on the DVE, trailing the exps
                    if jc == 0:
                        nc.vector.tensor_copy(acc[:], expT[:])
                    else:
                        nc.vector.tensor_add(acc[:], acc[:], expT[:])
                acc16 = acc_pool.tile([P, IB], F16, tag="acc16")
                nc.vector.tensor_copy(acc16[:], acc[:])

                for isub in range(NSUB):
                    pvs = [
                        pv_ps_pool.tile(
                            [P, 512], F32, tag="pv", name=f"pv_{ib}_{isub}_{eh}"
                        )
                        for eh in range(E // 512)
                    ]
                    # den[i] = sum_j exp[j,i]: DMA-transpose the summed exp
                    # slice to [i-partition, j-chunk] and reduce on the DVE —
                    # no PE involvement.
                    dent = dent_pool.tile([P, P], F16, tag="dent")
                    nc.sync.dma_start_transpose(
                        dent[:], acc16[:, isub * P : (isub + 1) * P]
                    )
                    den = small.tile([P, 1], F32, tag="den")
                    nc.vector.tensor_reduce(
                        den[:], dent[:],
                        axis=mybir.AxisListType.X, op=mybir.AluOpType.add,
                    )
                    for jc in range(NJC):
                        lhs = exps[jc][:, isub * P : (isub + 1) * P]
                        for eh in range(E // 512):
                            nc.tensor.matmul(
                                pvs[eh][:],
                                lhs,
                                v_sb[:, jc * E + eh * 512
                                     : jc * E + (eh + 1) * 512],
                                start=(jc == 0),
                                stop=(jc == NJC - 1),
                            )
                    recip = small.tile([P, 1], F32, tag="recip")
                    nc.vector.reciprocal(recip[:], den[:])
                    # Evictions split across DVE and ACT so they drain in
                    # parallel.
                    for eh in range(E // 512):
                        o = out_pool.tile([P, 512], F32, tag="o")
                        if eh == 0:
                            nc.vector.tensor_scalar_mul(
                                o[:], pvs[eh][:], recip[:]
                            )
                        else:
                            nc.scalar.activation(
                                o[:],
                                pvs[eh][:],
                                mybir.ActivationFunctionType.Copy,
                                scale=recip[:],
                            )
                        nc.sync.dma_start(
                            out_d.ap()[
                                i0 + isub * P : i0 + (isub + 1) * P,
                                eh * 512 : (eh + 1) * 512,
                            ],
                            o[:],
                        )

    nc.compile()
    return nc


def _get_nc():
    if "nc" not in _NC_CACHE:
        _NC_CACHE["nc"] = _build()
    return _NC_CACHE["nc"]


def kernel(q_in, k_v, Wq, bq, Wk, bk, Wv, bv):
    q_in = np.asarray(q_in, dtype=np.float32)
    k_v = np.asarray(k_v, dtype=np.float32)
    Wq32 = np.asarray(Wq, np.float32)
    Wk32 = np.asarray(Wk, np.float32)

    nc = _get_nc()

    # Weight folding: M = Wq Wk^T (fp32 accumulate, one fp16 rounding).
    M16 = np.ascontiguousarray((Wq32 @ Wk32.T).astype(np.float16))
    Wv16 = np.ascontiguousarray(np.asarray(Wv, np.float32).astype(np.float16))
    bv_bc = np.ascontiguousarray(
        np.broadcast_to(np.asarray(bv, np.float32), (P, E))
    )
    # Per-key logit bias from bq (zero when bq == 0); the dropped per-query
    # term is softmax-invariant.
    hvec = SCALE * (Wk32 @ np.asarray(bq, np.float32))

    in_maps = []
    for b in range(B):
        g = (k_v[b] @ hvec).astype(np.float32)  # [J], pre-scaled
        in_maps.append(
            {
                "q_inT": np.ascontiguousarray(q_in[b].T).astype(np.float16),
                "k_vT": np.ascontiguousarray(k_v[b].T).astype(np.float16),
                "M": M16,
                "Wv": Wv16,
                "bv_bc": bv_bc,
                "g_t": np.ascontiguousarray(g.reshape(NJC, P).T),
            }
        )

    global LAST_RESULTS
    LAST_RESULTS = run_bass_kernel_spmd(
        nc, in_maps, core_ids=list(range(B)), **_RUN_KWARGS
    )
    return np.stack([LAST_RESULTS.results[b]["out"] for b in range(B)])


# revision 29
# speedup vs baseline: 1.0141x; 1.0141x over previous
"""Cross-attention Trainium2 Bass kernel.

Problem (per full input):
    q_in [8, 2048, 1024] f32, k_v [8, 2048, 1024] f32,
    Wq/Wk/Wv [1024, 1024] f32, bq/bk/bv [1024] f32
    q = q_in @ Wq + bq; k = k_v @ Wk + bk; v = k_v @ Wv + bv
    out = softmax(q k^T / sqrt(1024)) v        -> [8, 2048, 1024] f32

Sharding: data-parallel over batch, one batch per NeuronCore (8 cores).

Weight folding (host, input-independent):
    q k^T = (q_in Wq + bq)(k_v Wk + bk)^T
          = q_in (Wq Wk^T) k_v^T  +  [per-i const: softmax-invariant, dropped]
          +  k_v (Wk bq)  +  [const: dropped]
  so with M = Wq Wk^T and g_j = SCALE * k_v[j] . (Wk bq):
    attn = softmax(SCALE * q' k_v^T + g),  q' = q_in M
  The k-projection disappears from the device entirely (2.1 GMAC/core saved)
  and the attention keys are the *raw* fp16 k_v (less accumulated rounding
  than a computed k).  M is rounded to fp16 once, same 3e-4 as any weight.

Per-core algorithm (I = J = 2048, E = D = 1024, P = 128):
  - Host pre-transposes the activations to [E, I]/[E, J] and casts to fp16.
  - q'-projection: q'T[d,i] computed with the M chunk stationary (output
    comes out transposed, exactly the layout the sim matmul needs);
    v[j,e] computed with the k_vT chunk stationary.
  - Attention: simT[j,i] = k_vT^T q'T accumulated over d in PSUM; exp on the
    ACT engine with the 1/sqrt(E) scale and per-j bias g fused; PV
    accumulates sum_j expT[j,i] v[j,e] over all j in PSUM (unnormalized);
    the softmax denominator is computed entirely off the PE (DVE partial
    sums of the exp tiles, a DMA transpose to [i,j] layout, and a DVE
    free-axis reduce); a per-partition reciprocal multiply normalizes at
    eviction.
  - exp is computed without max subtraction: sim ~ N(0,1) for this
    problem's distribution, so exp() stays comfortably inside fp16/fp32
    range and softmax is shift-invariant anyway.
"""

import numpy as np
from contextlib import ExitStack

import concourse.bass as bass
import concourse.mybir as mybir
import concourse.tile as tile
from concourse import bacc
from concourse.bass_utils import run_bass_kernel_spmd

B = 8
I = 2048  # query positions per batch
J = 2048  # key positions per batch
E = 1024  # embed dim
P = 128
EC = E // P  # 8 contraction chunks
NJC = J // P  # 16 key chunks
SCALE = float(E) ** -0.5

F16 = mybir.dt.float16
F32 = mybir.dt.float32

# i-block size for the attention phase (sim moving free dim).  512 (a full
# PSUM bank per sim tile) with the two-pass block structure: all 16 sim/exp
# tiles of a block first, then PV swept per i-subtile.  PSUM budget:
# 2 sim (double-buffer) + 4 PV (2 e-halves x 2 subtiles in flight) + 2 den.
IB = 512

# Module-level knobs test.py may override before the first kernel() call.
_RUN_KWARGS: dict = {}
LAST_RESULTS = None

_NC_CACHE: dict = {}


def _build():
    nc = bacc.Bacc("TRN2", target_bir_lowering=False, debug=False)

    q_inT = nc.dram_tensor("q_inT", [E, I], F16, kind="ExternalInput")
    k_vT = nc.dram_tensor("k_vT", [E, J], F16, kind="ExternalInput")
    M_d = nc.dram_tensor("M", [E, E], F16, kind="ExternalInput")
    Wv_d = nc.dram_tensor("Wv", [E, E], F16, kind="ExternalInput")
    bv_bc = nc.dram_tensor("bv_bc", [P, E], F32, kind="ExternalInput")
    g_d = nc.dram_tensor("g_t", [P, NJC], F32, kind="ExternalInput")
    out_d = nc.dram_tensor("out", [I, E], F32, kind="ExternalOutput")

    with tile.TileContext(nc) as tc, ExitStack() as ctx:
        const = ctx.enter_context(tc.tile_pool(name="const", bufs=1))
        bv_sb = const.tile([P, E], F32, tag="bv")
        g_sb = const.tile([P, NJC], F32, tag="g")

        # PE warmup: the HAM clock gate keeps the PE at 1.2 GHz until it has
        # seen ~3.4us of sustained matmul activity.  The first ~10us of the
        # kernel are DMA-bound anyway, so burn them on dummy matmuls over a
        # memset tile — by the time real operands land, the PE runs at 2.4.
        with ExitStack() as wu:
            wpool_ = wu.enter_context(tc.tile_pool(name="warm", bufs=1))
            wps_ = wu.enter_context(
                tc.tile_pool(name="warm_ps", bufs=1, space="PSUM")
            )
            wsrc = wpool_.tile([P, 256], F16)
            nc.vector.memset(wsrc[:], 0.0)
            wps = wps_.tile([P, 256], F32)
            for _ in range(20):
                nc.tensor.matmul(
                    wps[:], wsrc[:, :P], wsrc[:], start=True, stop=True
                )

        # Persistent fp16 operands for the attention phase.
        # q'T/k_vT: chunk d lives at [:, d*I + i]  (layout [d, i] / [d, j])
        # v:        chunk jc lives at [:, jc*E + e] (layout [j, e])
        persist = ctx.enter_context(tc.tile_pool(name="persist", bufs=1))
        qT_sb = persist.tile([P, EC * I], F16, tag="qT")
        kv_sb = persist.tile([P, EC * J], F16, tag="kv")
        v_sb = persist.tile([P, NJC * E], F16, tag="v")

        # Attention-phase SBUF pools are allocated up front, NOT scoped after
        # the projection pools: if they reused the weight/x-stream SBUF bytes
        # the allocator would add a WAR edge making the first exp wait on the
        # last v-projection matmul (~0.8us PE stall at the phase boundary).
        exp_pool = ctx.enter_context(tc.tile_pool(name="exp", bufs=18))
        out_pool = ctx.enter_context(tc.tile_pool(name="outsb", bufs=6))
        small = ctx.enter_context(tc.tile_pool(name="small", bufs=4))
        # Softmax-denominator staging: exp tiles are summed over key chunks
        # on the DVE, DMA-transposed to [i-partition, j] layout, and reduced
        # on the DVE — the PE never touches the denominator.
        acc_pool = ctx.enter_context(tc.tile_pool(name="dacc", bufs=2))
        dent_pool = ctx.enter_context(tc.tile_pool(name="dent", bufs=4))
        # sim PSUM pool allocated before the projection pool so it sits on
        # banks the projections never touch — the first sim matmuls at the
        # phase boundary then carry no PSUM WAR dependency on the last
        # v-projection evictions.
        sim_ps_pool = ctx.enter_context(
            tc.tile_pool(name="sim_ps", bufs=2, space="PSUM")
        )

        # ---------------- phase A/B: projections ----------------
        with ExitStack() as ab:
            wpool = ab.enter_context(tc.tile_pool(name="wpool", bufs=1))
            # Both weight matrices in one tile: matrix w chunk e at
            # [:, w*E*EC + e*E + d]   ([128, 16384] f16 = 32KB/partition).
            w_sb = wpool.tile([P, 2 * EC * E], F16, tag="W")
            w_off = {"M": 0, "Wv": EC * E}

            # NOTE: alternating input DMA dispatch between Sync and ACT (both
            # are HWDGE engines) measured 5us SLOWER than keeping everything
            # on Sync — don't split the input stream across queues.
            def in_dma(dst, src):
                nc.sync.dma_start(dst, src)

            def load_w_chunk(w, wd, e, dh_range=(0, 2)):
                # dh splits each weight chunk into d-halves so the DMA stream
                # can prioritize the columns the first PSUM groups need.
                for dh in range(*dh_range):
                    in_dma(
                        w_sb[:, w_off[w] + e * E + dh * 512
                             : w_off[w] + e * E + (dh + 1) * 512],
                        wd.ap()[e * P : (e + 1) * P, dh * 512 : (dh + 1) * 512],
                    )

            xpool = ab.enter_context(tc.tile_pool(name="xpool", bufs=2))
            ppool = ab.enter_context(
                tc.tile_pool(name="proj_ps", bufs=4, space="PSUM")
            )

            H = 1024  # half of the i range handled per streamed xT tile

            def load_q_half(h, with_m):
                # First half: DMA order matches the group order below — M
                # first-halves paired with the first 512-wide i-slice of each
                # q chunk (the exact data the first four PSUM groups need),
                # then the second i-slices, then the M second-halves.
                xh = xpool.tile([P, EC * H], F16, tag="xT")
                for e in range(EC):
                    if with_m:
                        load_w_chunk("M", M_d, e, dh_range=(0, 1))
                    in_dma(
                        xh[:, e * H : e * H + 512],
                        q_inT.ap()[e * P : (e + 1) * P, h * H : h * H + 512],
                    )
                    if h == 0 and with_m and e == 0:
                        in_dma(bv_sb[:], bv_bc.ap())
                        in_dma(g_sb[:], g_d.ap())
                for e in range(EC):
                    in_dma(
                        xh[:, e * H + 512 : (e + 1) * H],
                        q_inT.ap()[e * P : (e + 1) * P,
                                   h * H + 512 : (h + 1) * H],
                    )
                if with_m:
                    # second d-halves after the critical set
                    for e in range(EC):
                        load_w_chunk("M", M_d, e, dh_range=(1, 2))
                return xh

            def proj_q(xh, h, order):
                # q'T[d, n] = sum_e M[e,d] q_in[n,e], n in this half
                for d, ib in order:
                    ps = ppool.tile([P, 512], F32, tag="proj")
                    for e in range(EC):
                        nc.tensor.matmul(
                            ps[:],
                            w_sb[:, w_off["M"] + e * E + d * P
                                 : w_off["M"] + e * E + (d + 1) * P],
                            xh[:, e * H + ib * 512 : e * H + (ib + 1) * 512],
                            start=(e == 0),
                            stop=(e == EC - 1),
                        )
                    nc.scalar.activation(
                        qT_sb[:, d * I + h * H + ib * 512
                              : d * I + h * H + (ib + 1) * 512],
                        ps[:],
                        mybir.ActivationFunctionType.Copy,
                    )

            # first-half group order follows DMA arrival: (ib0, d<4) needs
            # only the M first-halves + first i-slices; (ib1, d<4) adds the
            # second i-slices; d>=4 waits on the M second-halves.
            order0 = ([(d, 0) for d in range(4)] + [(d, 1) for d in range(4)]
                      + [(d, ib) for d in range(4, EC) for ib in range(2)])
            order1 = [(d, ib) for d in range(EC) for ib in range(H // 512)]
            xh = load_q_half(0, True)
            proj_q(xh, 0, order0)
            xh = load_q_half(1, False)
            proj_q(xh, 1, order1)

            # keys (raw) + Wv stream in while the PE chews on q'
            for e in range(EC):
                in_dma(
                    kv_sb[:, e * J : (e + 1) * J],
                    k_vT.ap()[e * P : (e + 1) * P, :],
                )
                load_w_chunk("Wv", Wv_d, e)

            # v[j, e] = sum_e' k_v[j, e'] Wv[e', e] + bv[e]
            for jc in range(NJC):
                for eh in range(E // 512):
                    ps = ppool.tile([P, 512], F32, tag="proj")
                    for e in range(EC):
                        nc.tensor.matmul(
                            ps[:],
                            kv_sb[:, e * J + jc * P : e * J + (jc + 1) * P],
                            w_sb[:, w_off["Wv"] + e * E + eh * 512
                                 : w_off["Wv"] + e * E + (eh + 1) * 512],
                            start=(e == 0),
                            stop=(e == EC - 1),
                        )
                    nc.vector.tensor_add(
                        v_sb[:, jc * E + eh * 512 : jc * E + (eh + 1) * 512],
                        ps[:],
                        bv_sb[:, eh * 512 : (eh + 1) * 512],
                    )

        # ---------------- phase C: attention ----------------
        with ExitStack() as c:
            pv_ps_pool = c.enter_context(
                tc.tile_pool(name="pv_ps", bufs=4, space="PSUM")
            )
            NSUB = IB // P  # i-subtiles per block

            for ib in range(I // IB):
                i0 = ib * IB

                def emit_sim(jc):
                    sim = sim_ps_pool.tile([P, IB], F32, tag="sim",
                                           name=f"sim_{ib}_{jc}")
                    for d in range(EC):
                        nc.tensor.matmul(
                            sim[:],
                            kv_sb[:, d * J + jc * P : d * J + (jc + 1) * P],
                            qT_sb[:, d * I + i0 : d * I + i0 + IB],
                            start=(d == 0),
                            stop=(d == EC - 1),
                        )
                    return sim

                # All 16 sim tiles first (exp on ACT trails one tile behind),
                # then PV grouped by i-subtile so each subtile's
                # reciprocal+eviction overlaps the next subtile's PV matmuls.
                exps = []
                acc = acc_pool.tile([P, IB], F32, tag="acc")
                for jc in range(NJC):
                    sim = emit_sim(jc)
                    expT = exp_pool.tile([P, IB], F16, tag="expT")
                    nc.scalar.activation(
                        expT[:], sim[:], mybir.ActivationFunctionType.Exp,
                        scale=SCALE, bias=g_sb[:, jc : jc + 1],
                    )
                    exps.append(expT)
                    # denominator partial sum on the DVE, trailing the exps
                    if jc == 0:
                        nc.vector.tensor_copy(acc[:], expT[:])
                    else:
                        nc.vector.tensor_add(acc[:], acc[:], expT[:])
                acc16 = acc_pool.tile([P, IB], F16, tag="acc16")
                nc.vector.tensor_copy(acc16[:], acc[:])

                for isub in range(NSUB):
                    pvs = [
                        pv_ps_pool.tile(
                            [P, 512], F32, tag="pv", name=f"pv_{ib}_{isub}_{eh}"
                        )
                        for eh in range(E // 512)
                    ]
                    # den[i] = sum_j exp[j,i]: DMA-transpose the summed exp
                    # slice to [i-partition, j-chunk] and reduce on the DVE —
                    # no PE involvement.
                    dent = dent_pool.tile([P, P], F16, tag="dent")
                    nc.sync.dma_start_transpose(
                        dent[:], acc16[:, isub * P : (isub + 1) * P]
                    )
                    den = small.tile([P, 1], F32, tag="den")
                    nc.vector.tensor_reduce(
                        den[:], dent[:],
                        axis=mybir.AxisListType.X, op=mybir.AluOpType.add,
                    )
                    for jc in range(NJC):
                        lhs = exps[jc][:, isub * P : (isub + 1) * P]
                        for eh in range(E // 512):
                            nc.tensor.matmul(
                                pvs[eh][:],
                                lhs,
                                v_sb[:, jc * E + eh * 512
                                     : jc * E + (eh + 1) * 512],
                                start=(jc == 0),
                                stop=(jc == NJC - 1),
                            )
                    recip = small.tile([P, 1], F32, tag="recip")
                    nc.vector.reciprocal(recip[:], den[:])
                    # Evictions split across DVE and ACT so they drain in
                    # parallel.
                    for eh in range(E // 512):
                        o = out_pool.tile([P, 512], F32, tag="o")
                        if eh == 0:
                            nc.vector.tensor_scalar_mul(
                                o[:], pvs[eh][:], recip[:]
                            )
                        else:
                            nc.scalar.activation(
                                o[:],
                                pvs[eh][:],
                                mybir.ActivationFunctionType.Copy,
                                scale=recip[:],
                            )
                        nc.sync.dma_start(
                            out_d.ap()[
                                i0 + isub * P : i0 + (isub + 1) * P,
                                eh * 512 : (eh + 1) * 512,
                            ],
                            o[:],
                        )

    nc.compile()
    return nc


def _get_nc():
    if "nc" not in _NC_CACHE:
        _NC_CACHE["nc"] = _build()
    return _NC_CACHE["nc"]


def kernel(q_in, k_v, Wq, bq, Wk, bk, Wv, bv):
    q_in = np.asarray(q_in, dtype=np.float32)
    k_v = np.asarray(k_v, dtype=np.float32)
    Wq32 = np.asarray(Wq, np.float32)
    Wk32 = np.asarray(Wk, np.float32)

    nc = _get_nc()

    # Weight folding: M = Wq Wk^T (fp32 accumulate, one fp16 rounding).
    M16 = np.ascontiguousarray((Wq32 @ Wk32.T).astype(np.float16))
    Wv16 = np.ascontiguousarray(np.asarray(Wv, np.float32).astype(np.float16))
    bv_bc = np.ascontiguousarray(
        np.broadcast_to(np.asarray(bv, np.float32), (P, E))
    )
    # Per-key logit bias from bq (zero when bq == 0); the dropped per-query
    # term is softmax-invariant.
    hvec = SCALE * (Wk32 @ np.asarray(bq, np.float32))

    in_maps = []
    for b in range(B):
        g = (k_v[b] @ hvec).astype(np.float32)  # [J], pre-scaled
        in_maps.append(
            {
                "q_inT": np.ascontiguousarray(q_in[b].T).astype(np.float16),
                "k_vT": np.ascontiguousarray(k_v[b].T).astype(np.float16),
                "M": M16,
                "Wv": Wv16,
                "bv_bc": bv_bc,
                "g_t": np.ascontiguousarray(g.reshape(NJC, P).T),
            }
        )

    global LAST_RESULTS
    LAST_RESULTS = run_bass_kernel_spmd(
        nc, in_maps, core_ids=list(range(B)), **_RUN_KWARGS
    )
    return np.stack([LAST_RESULTS.results[b]["out"] for b in range(B)])
